# revision 7
# baseline (speedup 1.0000x reference)
"""Trainium2 Bass kernel for nn_Bottleneck_CSA_ConvBlock.

Computation (per image, C=64, H=W=160):
    y  = silu(bn1(conv3x3(x, w1)))
    fv = conv3x3(y, wv)
    k_sum = fk.sum(ch, h); f_scores[c] = scale * sum_hw fq[c,h,w]*k_sum[w]
    scores = softmax_c(f_scores)
    out = x + relu(bn2(scores*fv + y))

Matmul structure: each conv is lowered to 6 K=128 streams per 3-row output
block (3 dy-pair taps + 3 dy=2 singles with zeroed K-half), with the two
images of a core running concurrently in opposite PE column strips
(tile_position (0,0) / (0,64)).  K=128 comes from stacking the image and
its one-row-shifted copy on opposite partition halves:
    buf1: parts 0-63 = yA, parts 64-127 = yA shifted down one row
    buf2: parts 64-127 = yB, parts 0-63 = yB shifted (swapped halves)
The shifted copies are produced by per-chunk SBUF->SBUF DMAs; for x they
are loaded straight from HBM with a row offset.  fq/fk are never
materialized (scores reduce to functionals of y's column sums).  The
residual uses bf16 x (no f32 x load at all).

Sharding: pure data parallelism, 2 images per core across 8 cores.
"""

import numpy as np

C = 64
H = W = 160
HP = WP = 162          # padded
IMG = H * W            # 25600
PIMG = HP * WP         # 26244
BN_EPS = 1e-5

_CACHED = {}


def _build_nc(loop_iters=None):
    import concourse.bass as bass
    import concourse.tile as tile
    from concourse import bacc, mybir
    from concourse.masks import make_identity

    dt = mybir.dt
    AF = mybir.ActivationFunctionType
    AX = mybir.AxisListType
    f32 = dt.float32
    bf16 = dt.bfloat16

    nc = bacc.Bacc("TRN2", target_bir_lowering=False, debug=False, num_devices=8)

    xb_d = nc.dram_tensor("xbf", [128, IMG], bf16, kind="ExternalInput")
    w1_d = nc.dram_tensor("w1t", [128, 6, 128], bf16, kind="ExternalInput")
    wv_d = nc.dram_tensor("wvt", [128, 6, 128], bf16, kind="ExternalInput")
    wq_d = nc.dram_tensor("wqt", [128, 9, 65], bf16, kind="ExternalInput")
    bn1s_d = nc.dram_tensor("bn1s", [128, 1], f32, kind="ExternalInput")
    bn1b_d = nc.dram_tensor("bn1b", [128, 1], f32, kind="ExternalInput")
    bn2s_d = nc.dram_tensor("bn2s", [128, 1], f32, kind="ExternalInput")
    bn2b_d = nc.dram_tensor("bn2b", [128, 1], f32, kind="ExternalInput")
    out_d = nc.dram_tensor("out", [128, IMG], f32, kind="ExternalOutput")

    # output row blocks: (first interior row r0 in padded coords, n rows)
    blocks = [(1 + 3 * i, 3) for i in range(53)] + [(160, 1)]
    NCH = 27            # chunk capacity in padded rows (26 + 1 slack row)
    CH_LEN = 1 + NCH * WP + 4
    YLEN = 1 + 163 * WP + 4      # padded image + 1 extra garbage-safe row

    # stream s: s<3 -> dy-pair (0,1), dx=s, base row r0-1
    #           s>=3 -> dy=2 single, dx=s-3, base row r0+1
    def s_base(s, r0):
        return (r0 - 1) if s < 3 else (r0 + 1)

    def body(tc):
        with (
            tc.tile_pool(name="const", bufs=1) as const,
            tc.tile_pool(name="ybuf", bufs=1) as ybuf,
            tc.tile_pool(name="small", bufs=1) as small,
        ):
            w1_sb = const.tile([128, 6, 128], bf16)
            nc.sync.dma_start(out=w1_sb[:], in_=w1_d.ap())
            wv_sb = const.tile([128, 6, 128], bf16)
            nc.sync.dma_start(out=wv_sb[:], in_=wv_d.ap())
            wq_sb = const.tile([128, 9, 65], bf16)
            nc.sync.dma_start(out=wq_sb[:], in_=wq_d.ap())
            bn1s = const.tile([128, 1], f32)
            nc.sync.dma_start(out=bn1s[:], in_=bn1s_d.ap())
            bn1b = const.tile([128, 1], f32)
            nc.sync.dma_start(out=bn1b[:], in_=bn1b_d.ap())
            bn2s = const.tile([128, 1], f32)
            nc.sync.dma_start(out=bn2s[:], in_=bn2s_d.ap())
            bn2b = const.tile([128, 1], f32)
            nc.sync.dma_start(out=bn2b[:], in_=bn2b_d.ap())
            ident = const.tile([128, 128], f32)
            make_identity(nc, ident[:])
            ones_sb = const.tile([128, 64], bf16)
            nc.vector.memset(ones_sb[:], 1.0)

            # buf1: low = yA, high = yA<<WP ; buf2: low = yB<<WP, high = yB
            buf1 = ybuf.tile([128, YLEN], bf16)
            buf2 = ybuf.tile([128, YLEN], bf16)
            b1l = buf1[0:64, 1:1 + PIMG].rearrange("p (r c) -> p r c", c=WP)
            b2h = buf2[64:128, 1:1 + PIMG].rearrange("p (r c) -> p r c", c=WP)
            for bf, b3, lo in ((buf1, b1l, True), (buf2, b2h, False)):
                sl = bf[0:64] if lo else bf[64:128]
                nc.vector.memset(sl[:, 0:1], 0.0)
                nc.vector.memset(b3[:, 0, :], 0.0)          # top pad row
                nc.vector.memset(b3[:, HP - 1, :], 0.0)     # bottom pad row
                nc.vector.memset(b3[:, 1:HP - 1, 0:1], 0.0)
                nc.vector.memset(b3[:, 1:HP - 1, WP - 1:WP], 0.0)
                # tail slack + extra row (read by K-halves with zero weights,
                # must be finite)
                nc.vector.memset(sl[:, 1 + PIMG:YLEN], 0.0)
            # shifted halves: rows 161..162 beyond the last copied row
            nc.vector.memset(buf1[64:128, 1 + 161 * WP:YLEN], 0.0)
            nc.vector.memset(buf2[0:64, 1 + 161 * WP:YLEN], 0.0)

            Cs = small.tile([128, WP], f32)      # col sums (A low / B high)
            CmL = small.tile([128, WP], bf16)
            CmF = small.tile([128, WP], bf16)
            Cc = small.tile([128, WP], bf16)
            q0s = small.tile([65, 160], bf16)
            q1s = small.tile([65, 160], bf16)
            t0s = small.tile([64, 160], f32)
            t1s = small.tile([64, 160], f32)
            fs0 = small.tile([64, 1], f32)
            fs1 = small.tile([64, 1], f32)
            frow = small.tile([1, 128], f32)
            srow = small.tile([1, 128], f32)
            mx = small.tile([1, 1], f32, tag="mx")
            sm = small.tile([1, 1], f32, tag="sm")
            rs = small.tile([1, 1], f32, tag="rs")
            scores = small.tile([128, 1], f32)

            xbap = xb_d.ap()

            # ---------------- pass 1: conv1 -> y (+ shifted copies) ----------
            with (
                tc.tile_pool(name="chunks", bufs=2) as chunks,
                tc.tile_pool(name="ps1", bufs=6, space="PSUM") as ps1,
            ):
                for k in range(7):
                    pr0 = 24 * k
                    nrows = 26 if k < 6 else 18
                    chA = chunks.tile([128, CH_LEN], bf16, tag="chA")
                    chB = chunks.tile([128, CH_LEN], bf16, tag="chB")
                    A3 = chA[:, 1:1 + NCH * WP].rearrange("p (r c) -> p r c", c=WP)
                    B3 = chB[:, 1:1 + NCH * WP].rearrange("p (r c) -> p r c", c=WP)
                    for ch, c3 in ((chA, A3), (chB, B3)):
                        nc.vector.memset(ch[:, 0:1], 0.0)
                        nc.vector.memset(ch[:, 1 + nrows * WP:CH_LEN], 0.0)
                        nc.vector.memset(c3[:, 0:nrows, 0:1], 0.0)
                        nc.vector.memset(c3[:, 0:nrows, WP - 1:WP], 0.0)
                    # direct halves: local row l = padded row pr0+l
                    # shifted halves: local row l = padded row pr0+l+1
                    if k == 0:
                        d_l0, d_ir0, d_n = 1, 0, 25      # pad row at l=0
                        s_l0, s_ir0, s_n = 0, 0, 26
                        nc.vector.memset(A3[0:64, 0, :], 0.0)
                        nc.vector.memset(B3[64:128, 0, :], 0.0)
                    elif k < 6:
                        d_l0, d_ir0, d_n = 0, pr0 - 1, 26
                        s_l0, s_ir0, s_n = 0, pr0, 26
                    else:
                        d_l0, d_ir0, d_n = 0, pr0 - 1, 17   # pad at l=17
                        s_l0, s_ir0, s_n = 0, pr0, 16       # pads at l=16,17
                        nc.vector.memset(A3[0:64, 17, :], 0.0)
                        nc.vector.memset(B3[64:128, 17, :], 0.0)
                        nc.vector.memset(A3[64:128, 16:18, :], 0.0)
                        nc.vector.memset(B3[0:64, 16:18, :], 0.0)
                    # chunkA: low = xA direct, high = xA shifted
                    nc.sync.dma_start(
                        out=A3[0:64, d_l0:d_l0 + d_n, 1:1 + W],
                        in_=xbap[0:64, d_ir0 * W:(d_ir0 + d_n) * W].rearrange(
                            "p (r c) -> p r c", c=W))
                    nc.sync.dma_start(
                        out=A3[64:128, s_l0:s_l0 + s_n, 1:1 + W],
                        in_=xbap[0:64, s_ir0 * W:(s_ir0 + s_n) * W].rearrange(
                            "p (r c) -> p r c", c=W))
                    # chunkB: high = xB direct, low = xB shifted
                    nc.sync.dma_start(
                        out=B3[64:128, d_l0:d_l0 + d_n, 1:1 + W],
                        in_=xbap[64:128, d_ir0 * W:(d_ir0 + d_n) * W].rearrange(
                            "p (r c) -> p r c", c=W))
                    nc.sync.dma_start(
                        out=B3[0:64, s_l0:s_l0 + s_n, 1:1 + W],
                        in_=xbap[64:128, s_ir0 * W:(s_ir0 + s_n) * W].rearrange(
                            "p (r c) -> p r c", c=W))

                    for r0, nr in blocks:
                        if (r0 - 1) // 24 != k:
                            continue
                        N = nr * W
                        ps = ps1.tile([128, 3 * W], f32, tag="ps")
                        for s in range(6):
                            off = 1 + (s_base(s, r0) - pr0) * WP + (s % 3)
                            rhsA = chA[:, off:off + nr * WP].rearrange(
                                "p (r c) -> p r c", c=WP)[:, :, 0:W]
                            rhsB = chB[:, off:off + nr * WP].rearrange(
                                "p (r c) -> p r c", c=WP)[:, :, 0:W]
                            nc.tensor.matmul(
                                ps[0:64, :N], w1_sb[:, s, 0:64], rhsA,
                                start=(s == 0), stop=(s == 5),
                                tile_position=(0, 0))
                            nc.tensor.matmul(
                                ps[64:128, :N], w1_sb[:, s, 64:128], rhsB,
                                start=(s == 0), stop=(s == 5),
                                tile_position=(0, 64))
                        nc.scalar.activation(
                            out=b1l[:, r0:r0 + nr, 1:1 + W],
                            in_=ps[0:64, :N],
                            func=AF.Silu, bias=bn1b[0:64], scale=bn1s[0:64])
                        nc.scalar.activation(
                            out=b2h[:, r0:r0 + nr, 1:1 + W],
                            in_=ps[64:128, :N],
                            func=AF.Silu, bias=bn1b[64:128], scale=bn1s[64:128])

                    # shifted-copy DMAs for this chunk's fresh rows
                    yr0 = 24 * k + 1
                    ynr = 24 if k < 6 else 16
                    cnr = ynr if k < 6 else 17        # include bottom pad row
                    nc.sync.dma_start(
                        out=buf1[64:128, 1 + (yr0 - 1) * WP:
                                 1 + (yr0 - 1 + cnr) * WP],
                        in_=buf1[0:64, 1 + yr0 * WP:1 + (yr0 + cnr) * WP])
                    nc.sync.dma_start(
                        out=buf2[0:64, 1 + (yr0 - 1) * WP:
                                 1 + (yr0 - 1 + cnr) * WP],
                        in_=buf2[64:128, 1 + yr0 * WP:1 + (yr0 + cnr) * WP])

                    # partial column sums over freshly written rows
                    part = small.tile([128, WP], f32, tag="part")
                    nc.vector.reduce_sum(
                        part[0:64, :],
                        b1l[:, yr0:yr0 + ynr, :].rearrange("p r c -> p c r"),
                        axis=AX.X)
                    nc.vector.reduce_sum(
                        part[64:128, :],
                        b2h[:, yr0:yr0 + ynr, :].rearrange("p r c -> p c r"),
                        axis=AX.X)
                    if k == 0:
                        nc.vector.tensor_copy(Cs[:], part[:])
                    else:
                        nc.vector.tensor_add(Cs[:], Cs[:], part[:])

            # ---------------- scores (small path) ----------------
            with tc.tile_pool(name="ps_s", bufs=2, space="PSUM") as pss:
                nc.vector.tensor_sub(CmL[0:64, :], Cs[0:64, :], b1l[:, H, :])
                nc.vector.tensor_sub(CmF[0:64, :], Cs[0:64, :], b1l[:, 1, :])
                nc.vector.tensor_sub(CmL[64:128, :], Cs[64:128, :], b2h[:, H, :])
                nc.vector.tensor_sub(CmF[64:128, :], Cs[64:128, :], b2h[:, 1, :])
                nc.vector.tensor_copy(Cc[:], Cs[:])
                s_of = {0: CmL, 1: Cc, 2: CmF}

                qp0 = pss.tile([65, 160], f32, tag="qp")
                qp1 = pss.tile([65, 160], f32, tag="qp")
                for k9 in range(9):
                    dy, dx = divmod(k9, 3)
                    nc.tensor.matmul(
                        qp0[:, :], wq_sb[0:64, k9, :],
                        s_of[dy][0:64, dx:dx + 160],
                        start=(k9 == 0), stop=(k9 == 8), tile_position=(0, 0))
                for k9 in range(9):
                    dy, dx = divmod(k9, 3)
                    nc.tensor.matmul(
                        qp1[:, :], wq_sb[64:128, k9, :],
                        s_of[dy][64:128, dx:dx + 160],
                        start=(k9 == 0), stop=(k9 == 8), tile_position=(64, 0))
                nc.vector.tensor_copy(q0s[:], qp0[:])
                nc.vector.tensor_copy(q1s[:], qp1[:])

                # broadcast k_sum row (partition 64) across 64 partitions
                bc0 = pss.tile([64, 160], f32, tag="bc")
                bc1 = pss.tile([64, 160], f32, tag="bc")
                nc.tensor.matmul(bc0[:, :], ones_sb[64:65, :], q0s[64:65, :],
                                 start=True, stop=True, tile_position=(64, 0))
                nc.tensor.matmul(bc1[:, :], ones_sb[64:65, :], q1s[64:65, :],
                                 start=True, stop=True, tile_position=(64, 0))
                nc.vector.tensor_mul(t0s[:], q0s[0:64, :], bc0[:])
                nc.vector.tensor_mul(t1s[:], q1s[0:64, :], bc1[:])
                nc.vector.reduce_sum(fs0[:], t0s[:], axis=AX.X)
                nc.vector.reduce_sum(fs1[:], t1s[:], axis=AX.X)

                tr0 = pss.tile([1, 64], f32, tag="tr")
                tr1 = pss.tile([1, 64], f32, tag="tr")
                nc.tensor.transpose(tr0[:], fs0[:], ident[0:64, 0:64])
                nc.tensor.transpose(tr1[:], fs1[:], ident[0:64, 0:64])
                nc.vector.tensor_copy(frow[0:1, 0:64], tr0[:])
                nc.vector.tensor_copy(frow[0:1, 64:128], tr1[:])

                for img in range(2):
                    seg = frow[0:1, 64 * img:64 * img + 64]
                    oseg = srow[0:1, 64 * img:64 * img + 64]
                    nc.vector.reduce_max(mx[:], seg, axis=AX.X, negate=True)
                    nc.scalar.activation(out=oseg, in_=seg, func=AF.Exp,
                                         bias=mx[:], scale=1.0)
                    nc.vector.reduce_sum(sm[:], oseg, axis=AX.X)
                    nc.vector.reciprocal(rs[:], sm[:])
                    nc.vector.tensor_scalar_mul(oseg, oseg, rs[:])

                psc = pss.tile([128, 1], f32, tag="psc")
                nc.tensor.transpose(psc[:], srow[:], ident[0:1, 0:1])
                nc.vector.tensor_copy(scores[:], psc[:])

            # ---------------- pass 2: conv_v -> epilogue -> out ----------------
            with (
                tc.tile_pool(name="ps2", bufs=6, space="PSUM") as ps2,
                tc.tile_pool(name="epi", bufs=3) as epi,
            ):
                for r0, nr in blocks:
                    M = nr * W
                    ps = ps2.tile([128, 3 * W], f32, tag="ps")
                    for s in range(6):
                        off = 1 + s_base(s, r0) * WP + (s % 3)
                        rhsA = buf1[:, off:off + nr * WP].rearrange(
                            "p (r c) -> p r c", c=WP)[:, :, 0:W]
                        rhsB = buf2[:, off:off + nr * WP].rearrange(
                            "p (r c) -> p r c", c=WP)[:, :, 0:W]
                        nc.tensor.matmul(
                            ps[0:64, :M], wv_sb[:, s, 0:64], rhsA,
                            start=(s == 0), stop=(s == 5),
                            tile_position=(0, 0))
                        nc.tensor.matmul(
                            ps[64:128, :M], wv_sb[:, s, 64:128], rhsB,
                            start=(s == 0), stop=(s == 5),
                            tile_position=(0, 64))
                    u = epi.tile([128, 3 * W], bf16, tag="u")
                    nc.scalar.mul(u[:, :M], ps[:, :M], scores[:])
                    u2 = epi.tile([128, 3 * W], bf16, tag="u2")
                    nc.vector.tensor_add(u2[0:64, :M], u[0:64, :M],
                                         b1l[:, r0:r0 + nr, 1:1 + W])
                    nc.vector.tensor_add(u2[64:128, :M], u[64:128, :M],
                                         b2h[:, r0:r0 + nr, 1:1 + W])
                    rt = epi.tile([128, 3 * W], bf16, tag="rt")
                    nc.scalar.activation(out=rt[:, :M], in_=u2[:, :M],
                                         func=AF.Relu, bias=bn2b[:], scale=bn2s[:])
                    xt = epi.tile([128, 3 * W], bf16, tag="xt")
                    nc.sync.dma_start(out=xt[:, :M],
                                      in_=xbap[:, (r0 - 1) * W:(r0 - 1) * W + M])
                    ot = epi.tile([128, 3 * W], f32, tag="ot")
                    nc.vector.tensor_add(ot[:, :M], rt[:, :M], xt[:, :M])
                    nc.sync.dma_start(out=out_d.ap()[:, (r0 - 1) * W:(r0 - 1) * W + M],
                                      in_=ot[:, :M])

    with tile.TileContext(nc) as tc:
        ctx_lp = nc.allow_low_precision("bf16 matmul path; fp32 PSUM accumulation")
        ctx_lp.__enter__()
        if loop_iters is None:
            body(tc)
        else:
            with tc.For_i(0, loop_iters):
                body(tc)
        ctx_lp.__exit__(None, None, None)
    nc.compile()
    return nc


def _get_nc():
    if "nc" not in _CACHED:
        _CACHED["nc"] = _build_nc()
    return _CACHED["nc"]


def _prep_weights(w_cv1, wq, wk, wv, g1, b1, m1, v1, g2, b2, m2, v2):
    import ml_dtypes
    bf = ml_dtypes.bfloat16

    def wpair(w):  # [co, ci, ky, kx] -> [128, 6, 128] (strip A | strip B)
        t = w.transpose(1, 2, 3, 0)                  # [ci, ky, kx, co]
        A = np.zeros((128, 6, 64), np.float32)
        B = np.zeros((128, 6, 64), np.float32)
        for dx in range(3):
            A[0:64, dx] = t[:, 0, dx]       # K-low  = direct  -> dy 0
            A[64:128, dx] = t[:, 1, dx]     # K-high = shifted -> dy 1
            A[0:64, 3 + dx] = t[:, 2, dx]   # singles: dy 2 on direct half
            B[64:128, dx] = t[:, 0, dx]     # strip B: halves swapped
            B[0:64, dx] = t[:, 1, dx]
            B[64:128, 3 + dx] = t[:, 2, dx]
        return np.ascontiguousarray(
            np.concatenate([A, B], axis=2).astype(bf))

    w1t = wpair(w_cv1)
    wvt = wpair(wv)

    scale = 1.0 / (float(W) ** 0.5 * float(H) * float(H))
    q = wq.transpose(1, 2, 3, 0).reshape(C, 9, C) * scale    # [j, 9, c]
    ks = wk.sum(axis=0).reshape(C, 9, 1)                     # [j, 9, 1]
    qa = np.concatenate([q, ks], axis=2)                     # [j, 9, 65]
    wqt = np.ascontiguousarray(np.concatenate([qa, qa], axis=0).astype(bf))

    s1 = (g1 / np.sqrt(v1 + BN_EPS)).astype(np.float32)
    b1p = (b1 - m1 * s1).astype(np.float32)
    s2 = (g2 / np.sqrt(v2 + BN_EPS)).astype(np.float32)
    b2p = (b2 - m2 * s2).astype(np.float32)

    def dup(v):
        return np.ascontiguousarray(
            np.concatenate([v, v]).reshape(128, 1).astype(np.float32))

    return dict(w1t=w1t, wvt=wvt, wqt=wqt,
                bn1s=dup(s1), bn1b=dup(b1p), bn2s=dup(s2), bn2b=dup(b2p))


def _ensure_axon_devices():
    """Make sure jax can see the 8 axon-tunneled NeuronCores even if the
    calling process pinned JAX_PLATFORMS=cpu before importing us."""
    import os
    envp = os.environ.get("JAX_PLATFORMS", "")
    if envp and "axon" not in envp:
        os.environ.pop("JAX_PLATFORMS", None)
    import jax
    try:
        devs = jax.devices()
        if len(devs) >= 8 and all("cpu" not in str(d).lower() for d in devs[:8]):
            return
    except Exception:
        pass
    try:
        from jax._src import xla_bridge
        xla_bridge.backends.cache_clear()
    except Exception:
        pass
    try:
        import jax.extend.backend as jeb
        jeb.clear_backends()
    except Exception:
        pass


def kernel(x, w_cv1, g1, b1, m1, v1, wq, wk, wv, g2, b2, m2, v2):
    import ml_dtypes
    _ensure_axon_devices()
    from concourse.bass_utils import run_bass_kernel_spmd

    x = np.asarray(x, dtype=np.float32)
    consts = _prep_weights(
        np.asarray(w_cv1, np.float32), np.asarray(wq, np.float32),
        np.asarray(wk, np.float32), np.asarray(wv, np.float32),
        np.asarray(g1, np.float32), np.asarray(b1, np.float32),
        np.asarray(m1, np.float32), np.asarray(v1, np.float32),
        np.asarray(g2, np.float32), np.asarray(b2, np.float32),
        np.asarray(m2, np.float32), np.asarray(v2, np.float32))
    nc = _get_nc()
    in_maps = []
    for i in range(8):
        xi = np.ascontiguousarray(x[2 * i:2 * i + 2].reshape(128, IMG))
        m = {"xbf": np.ascontiguousarray(xi.astype(ml_dtypes.bfloat16))}
        m.update(consts)
        in_maps.append(m)
    res = run_bass_kernel_spmd(nc, in_maps, core_ids=list(range(8)))
    outs = [r["out"].reshape(2, C, H, W) for r in res.results]
    return np.concatenate(outs, axis=0).astype(np.float32)
